# revision 6
# baseline (speedup 1.0000x reference)
"""Trainium2 Bass kernel for a dense transformer block (B=2, T=2048, C=1024,
H=16, HS=64, FF=4096, fp32, causal attention, scale=C**-0.5).

Sharding over 8 NeuronCores: core c -> batch g=c//4, rank r=c%4.
 - Attention: head-parallel (4 heads per core) in transposed-activation layout.
 - AllGather of per-head attention outputs within each 4-core batch group.
 - proj / LN2 / FFN: token-parallel (512 tokens per core); the rank-dependent
   token window of the gathered attention tensor is selected with a single
   partition_id-driven dynamic DMA so the program is identical on all cores.
All matmuls run as float32r (full fp32 data, 1 cycle/row at N>=256).
"""

import sys

import numpy as np

for _p in ("/opt/trn_rl_repo", "/root/.axon_site/_ro/trn_rl_repo"):
    if _p not in sys.path:
        sys.path.append(_p)

import concourse.bass as bass
import concourse.mybir as mybir
import concourse.tile as tile
from concourse import bacc
from concourse.bass_utils import run_bass_kernel_spmd
from concourse.masks import make_identity

P = 128
B, T, C, H, HS, FF = 2, 2048, 1024, 16, 64, 4096
EPS = 1e-5
NCORES = 8
GSZ = 4            # cores per batch group
NHL = H // GSZ     # 4 local heads per core
TLOC = T // GSZ    # 512 tokens per core
KT = C // P        # 8 contraction tiles over C
NTT = T // P       # 16 token tiles
NJ = T // 512      # 4 q-chunks of 512
MTL = TLOC // P    # 4 local token tiles
MFF = FF // P      # 32 ff tiles
KF = FF // P       # 32 contraction tiles over FF

F32 = mybir.dt.float32
F32R = mybir.dt.float32r
REPLICA_GROUPS = [[0, 1, 2, 3], [4, 5, 6, 7]]


def _mm(nc, out, lhsT, rhs, start, stop):
    nc.tensor.matmul(
        out, lhsT.bitcast(F32R), rhs.bitcast(F32R), start=start, stop=stop
    )


def _ln_stats(nc, pool, x_t):
    """mean/var over the free dim (C) of x_t [P, C] -> (rstd [P,1], nmr [P,1])
    with nmr = -mean*rstd."""
    xg = x_t.rearrange("p (s d) -> p s d", d=512)
    nsub = xg.shape[1]
    stats = pool.tile([P, nsub, 6], F32, tag="ln_stats", name="ln_stats")
    for s in range(nsub):
        nc.vector.bn_stats(out=stats[:, s, :], in_=xg[:, s, :])
    mv = pool.tile([P, 2], F32, tag="ln_mv", name="ln_mv")
    nc.vector.bn_aggr(out=mv[:], in_=stats[:])
    eps_t = pool.tile([P, 1], F32, tag="ln_eps", name="ln_eps")
    nc.vector.memset(eps_t[:], EPS)
    rstd = pool.tile([P, 1], F32, tag="ln_rstd", name="ln_rstd")
    nc.scalar.activation(
        out=rstd[:], in_=mv[:, 1:2], func=mybir.ActivationFunctionType.Sqrt,
        bias=eps_t[:],
    )
    nc.vector.reciprocal(out=rstd[:], in_=rstd[:])
    nmr = pool.tile([P, 1], F32, tag="ln_nmr", name="ln_nmr")
    nc.vector.tensor_mul(nmr[:], mv[:, 0:1], rstd[:])
    nc.vector.tensor_scalar_mul(nmr[:], nmr[:], -1.0)
    return rstd, nmr


def _emit(tc, io):
    from contextlib import ExitStack

    nc = tc.nc
    ID = mybir.ActivationFunctionType.Identity
    EXP = mybir.ActivationFunctionType.Exp
    RELU = mybir.ActivationFunctionType.Relu

    with ExitStack() as top:
        consts = top.enter_context(tc.tile_pool(name="consts", bufs=1))
        dram = top.enter_context(tc.tile_pool(name="dram", bufs=1, space="DRAM"))
        stat_pool = top.enter_context(tc.tile_pool(name="stats", bufs=4))

        ident = consts.tile([P, P], F32)
        make_identity(nc, ident)

        hT_dram = dram.tile([C, T], F32)
        ag_in = dram.tile([NHL * HS, T], F32)
        ag_out = dram.tile([C, T], F32)

        # ---------- Phase A: ln1(x) -> hT (transposed, via DRAM) ----------
        with ExitStack() as ph:
            xpool = ph.enter_context(tc.tile_pool(name="xin", bufs=3))
            hpool = ph.enter_context(tc.tile_pool(name="happ", bufs=3))
            tpsum = ph.enter_context(tc.tile_pool(name="tpsum", bufs=4, space="PSUM"))
            ttmp = ph.enter_context(tc.tile_pool(name="ttmp", bufs=4))
            for ti in range(NTT):
                x_t = xpool.tile([P, C], F32, tag="x_t", name="x_t")
                nc.sync.dma_start(x_t[:], io["x_full"][ti * P:(ti + 1) * P, :])
                rstd, nmr = _ln_stats(nc, stat_pool, x_t)
                h_t = hpool.tile([P, C], F32, tag="h_t", name="h_t")
                nc.scalar.activation(
                    out=h_t[:], in_=x_t[:], func=ID, bias=nmr[:], scale=rstd[:]
                )
                for kt in range(KT):
                    ps = tpsum.tile([P, P], F32, tag="tp", name="tp")
                    nc.tensor.transpose(ps[:], h_t[:, kt * P:(kt + 1) * P], ident[:])
                    tmp = ttmp.tile([P, P], F32, tag="tt", name="tt")
                    nc.scalar.copy(tmp[:], ps[:])
                    nc.sync.dma_start(
                        hT_dram[kt * P:(kt + 1) * P, ti * P:(ti + 1) * P], tmp[:]
                    )

        # ---------- Phases B+C: QKV + attention ----------
        with ExitStack() as att_stack:
            attw = att_stack.enter_context(tc.tile_pool(name="attw", bufs=1))
            big = att_stack.enter_context(tc.tile_pool(name="attbig", bufs=1))

            wq_sb = attw.tile([P, KT, 2, P], F32R)
            nc.sync.dma_start(wq_sb[:], io["wq"][:].bitcast(F32R))
            wk_sb = attw.tile([P, KT, 2, P], F32R)
            nc.sync.dma_start(wk_sb[:], io["wk"][:].bitcast(F32R))
            wv_sb = attw.tile([P, KT, NHL * HS], F32R)
            nc.sync.dma_start(wv_sb[:], io["wv"][:].bitcast(F32R))
            bq_sb = attw.tile([P, 2], F32)
            nc.sync.dma_start(bq_sb[:], io["bq"][:])
            bk_sb = attw.tile([P, 2], F32)
            nc.sync.dma_start(bk_sb[:], io["bk"][:])
            bvbc = attw.tile([P, NHL * HS], F32)
            nc.sync.dma_start(bvbc[:], io["bv"][None, :].to_broadcast((P, NHL * HS)))

            # causal masks for the 4 diagonal offsets: keep where q >= k,
            # i.e. (col + 512J) - (row + 128i) >= 0 with d = i - 4J.
            masks = []
            for d in range(4):
                m = attw.tile([P, 512], F32, tag=f"mask{d}", name=f"mask{d}")
                nc.gpsimd.memset(m[:], 0.0)
                nc.gpsimd.affine_select(
                    out=m[:], in_=m[:], compare_op=mybir.AluOpType.is_ge,
                    fill=-1e9, base=-128 * d, pattern=[[1, 512]],
                    channel_multiplier=-1,
                )
                masks.append(m)

            qT_sb = big.tile([P, 2, T], F32R)
            kT_sb = big.tile([P, 2, T], F32R)
            v_sb = big.tile([P, NTT, NHL, HS + 1], F32R)
            ones_c = attw.tile([P, NHL, 1], F32)
            nc.vector.memset(ones_c[:], 1.0)

            # ----- Phase B: QKV -----
            with ExitStack() as ph:
                jpool = ph.enter_context(tc.tile_pool(name="hTj", bufs=2))
                qkpsum = ph.enter_context(
                    tc.tile_pool(name="qkpsum", bufs=3, space="PSUM")
                )
                hT_view = hT_dram[:].bitcast(F32R).rearrange(
                    "(kt p) t -> p kt t", p=P
                )
                for J in range(NJ):
                    hTJ = jpool.tile([P, KT, 512], F32R, tag="hTJ", name="hTJ")
                    nc.sync.dma_start(hTJ[:], hT_view[:, :, J * 512:(J + 1) * 512])
                    for m in range(4):
                        mt = 4 * J + m
                        vps = qkpsum.tile([P, 512], F32, tag="qkv_ps", name="vps")
                        for kt in range(KT):
                            _mm(nc, vps[:, :NHL * HS],
                                hTJ[:, kt, m * P:(m + 1) * P],
                                wv_sb[:, kt, :], kt == 0, kt == KT - 1)
                        for h in range(NHL):
                            nc.vector.tensor_add(
                                v_sb[:, mt, h, 0:HS],
                                vps[:, h * HS:(h + 1) * HS],
                                bvbc[:, h * HS:(h + 1) * HS],
                            )
                        nc.vector.tensor_copy(
                            v_sb[:, mt, :, HS:HS + 1], ones_c[:]
                        )
                    for pair in range(2):
                        qps = qkpsum.tile([P, 512], F32, tag="qkv_ps", name="qps")
                        for kt in range(KT):
                            _mm(nc, qps[:], wq_sb[:, kt, pair, :], hTJ[:, kt, :],
                                kt == 0, kt == KT - 1)
                        nc.scalar.activation(
                            out=qT_sb[:, pair, J * 512:(J + 1) * 512], in_=qps[:],
                            func=ID, bias=bq_sb[:, pair:pair + 1],
                        )
                        kps = qkpsum.tile([P, 512], F32, tag="qkv_ps", name="kps")
                        for kt in range(KT):
                            _mm(nc, kps[:], wk_sb[:, kt, pair, :], hTJ[:, kt, :],
                                kt == 0, kt == KT - 1)
                        nc.scalar.activation(
                            out=kT_sb[:, pair, J * 512:(J + 1) * 512], in_=kps[:],
                            func=ID, bias=bk_sb[:, pair:pair + 1],
                        )

            # ----- Phase C: causal attention per (head, q-chunk) -----
            with ExitStack() as ph:
                stpsum = ph.enter_context(
                    tc.tile_pool(name="stpsum", bufs=3, space="PSUM")
                )
                upsum = ph.enter_context(
                    tc.tile_pool(name="upsum", bufs=2, space="PSUM")
                )
                ppool = ph.enter_context(tc.tile_pool(name="pT", bufs=4))
                mpool = ph.enter_context(tc.tile_pool(name="mtmp", bufs=3))
                npool = ph.enter_context(tc.tile_pool(name="norm", bufs=3))
                for h in range(NHL):
                    pair, off = h // 2, HS * (h % 2)
                    for J in range(NJ):
                        nk = 4 * J + 4
                        ups = upsum.tile([P, 512], F32, tag="ups", name="ups")
                        for i in range(nk):
                            sps = stpsum.tile([P, 512], F32, tag="sps", name="sps")
                            _mm(nc, sps[:],
                                kT_sb[off:off + HS, pair, i * P:(i + 1) * P],
                                qT_sb[off:off + HS, pair, J * 512:(J + 1) * 512],
                                True, True)
                            pT = ppool.tile([P, 512], F32R, tag="pT", name="pT")
                            if i >= 4 * J:
                                tmp = mpool.tile([P, 512], F32, tag="mtmp",
                                                 name="mtmp")
                                nc.vector.tensor_add(
                                    tmp[:], sps[:], masks[i - 4 * J][:]
                                )
                                nc.scalar.activation(out=pT[:], in_=tmp[:], func=EXP)
                            else:
                                nc.scalar.activation(out=pT[:], in_=sps[:], func=EXP)
                            _mm(nc, ups[:HS + 1, :], v_sb[:, i, h, :], pT[:],
                                i == 0, i == nk - 1)
                        recip = npool.tile([1, 512], F32, tag="recip", name="recip")
                        nc.vector.reciprocal(recip[:], ups[HS:HS + 1, :])
                        rbc = npool.tile([HS, 512], F32, tag="rbc", name="rbc")
                        nc.gpsimd.partition_broadcast(rbc[:], recip[:])
                        att = npool.tile([HS, 512], F32, tag="att", name="att")
                        nc.vector.tensor_mul(att[:], ups[0:HS, :], rbc[:])
                        nc.sync.dma_start(
                            ag_in[h * HS:(h + 1) * HS, J * 512:(J + 1) * 512],
                            att[:],
                        )

        # ---------- Phase D..G persistent tiles ----------
        with ExitStack() as tail:
            tailp = tail.enter_context(tc.tile_pool(name="tailp", bufs=1))
            y_sb = tailp.tile([P, MTL, C], F32)

            # ----- Phase D: AllGather + proj + residual -----
            nc.gpsimd.collective_compute(
                "AllGather", mybir.AluOpType.bypass,
                replica_groups=REPLICA_GROUPS,
                ins=[ag_in[:].opt()], outs=[ag_out[:].opt()],
            )
            with ExitStack() as ph:
                agp = ph.enter_context(tc.tile_pool(name="agp", bufs=1))
                prpsum = ph.enter_context(
                    tc.tile_pool(name="prpsum", bufs=3, space="PSUM")
                )
                xpb_sb = agp.tile([P, MTL, C], F32)
                nc.sync.dma_start(xpb_sb[:], io["xpb"][:])
                wo_sb = agp.tile([P, KT, C], F32R)
                nc.sync.dma_start(wo_sb[:], io["wo"][:].bitcast(F32R))
                pid = nc.sync.partition_id()
                tok0 = (pid % GSZ) * TLOC
                ag_sb = agp.tile([P, KT, TLOC], F32R)
                nc.sync.dma_start(
                    ag_sb[:],
                    ag_out[:].bitcast(F32R).rearrange("(kt p) t -> p kt t", p=P)[
                        :, :, bass.ds(tok0, TLOC)
                    ],
                )
                for mt in range(MTL):
                    for nt in range(2):
                        pps = prpsum.tile([P, 512], F32, tag="pps", name="pps")
                        for kt in range(KT):
                            _mm(nc, pps[:], ag_sb[:, kt, mt * P:(mt + 1) * P],
                                wo_sb[:, kt, nt * 512:(nt + 1) * 512],
                                kt == 0, kt == KT - 1)
                        nc.vector.tensor_add(
                            y_sb[:, mt, nt * 512:(nt + 1) * 512], pps[:],
                            xpb_sb[:, mt, nt * 512:(nt + 1) * 512],
                        )

            yT_sb = tailp.tile([P, KT, TLOC], F32R)
            rT = tailp.tile([P, MFF, TLOC], F32R)
            b1p_sb = tailp.tile([P, MFF], F32)
            nc.sync.dma_start(b1p_sb[:], io["b1p"][:])
            b2bc = tailp.tile([P, C], F32)
            nc.sync.dma_start(b2bc[:], io["b2"][None, :].to_broadcast((P, C)))

            # ----- Phase E: ln2 + transpose -----
            with ExitStack() as ph:
                yhp = ph.enter_context(tc.tile_pool(name="yh", bufs=2))
                tp2 = ph.enter_context(tc.tile_pool(name="tp2", bufs=4, space="PSUM"))
                for mt in range(MTL):
                    rstd, nmr = _ln_stats(nc, stat_pool, y_sb[:, mt, :])
                    yh = yhp.tile([P, C], F32, tag="yh", name="yh")
                    nc.scalar.activation(
                        out=yh[:], in_=y_sb[:, mt, :], func=ID, bias=nmr[:],
                        scale=rstd[:],
                    )
                    for kt in range(KT):
                        ps = tp2.tile([P, P], F32, tag="tp2", name="tp2")
                        nc.tensor.transpose(
                            ps[:], yh[:, kt * P:(kt + 1) * P], ident[:]
                        )
                        nc.scalar.copy(yT_sb[:, kt, mt * P:(mt + 1) * P], ps[:])

            # ----- Phase F: FFN1 (relu(yT @ W1 + b1) -> rT) -----
            with ExitStack() as ph:
                w1p = ph.enter_context(tc.tile_pool(name="w1p", bufs=3))
                zps_p = ph.enter_context(
                    tc.tile_pool(name="zps", bufs=3, space="PSUM")
                )
                w1_view = io["w1"].bitcast(F32R).rearrange(
                    "(kt p) f -> p kt f", p=P
                )
                for mf in range(MFF):
                    w1_t = w1p.tile([P, KT, P], F32R, tag="w1t", name="w1t")
                    nc.sync.dma_start(w1_t[:], w1_view[:, :, mf * P:(mf + 1) * P])
                    zps = zps_p.tile([P, 512], F32, tag="zps", name="zps")
                    for kt in range(KT):
                        _mm(nc, zps[:], w1_t[:, kt, :], yT_sb[:, kt, :],
                            kt == 0, kt == KT - 1)
                    nc.scalar.activation(
                        out=rT[:, mf, :], in_=zps[:], func=RELU,
                        bias=b1p_sb[:, mf:mf + 1],
                    )

            # ----- Phase G: FFN2 + residual + out -----
            with ExitStack() as ph:
                w2p = ph.enter_context(tc.tile_pool(name="w2p", bufs=3))
                fps_p = ph.enter_context(
                    tc.tile_pool(name="fps", bufs=1, space="PSUM")
                )
                otmp = ph.enter_context(tc.tile_pool(name="otmp", bufs=3))
                fps = [
                    [
                        fps_p.tile(
                            [P, 512], F32, tag=f"fps_{mt}_{nt}",
                            name=f"fps_{mt}_{nt}",
                        )
                        for nt in range(2)
                    ]
                    for mt in range(MTL)
                ]
                for kt in range(KF):
                    w2_t = w2p.tile([P, C], F32R, tag="w2t", name="w2t")
                    nc.sync.dma_start(
                        w2_t[:], io["w2"][kt * P:(kt + 1) * P, :].bitcast(F32R)
                    )
                    for mt in range(MTL):
                        for nt in range(2):
                            _mm(nc, fps[mt][nt][:],
                                rT[:, kt, mt * P:(mt + 1) * P],
                                w2_t[:, nt * 512:(nt + 1) * 512],
                                kt == 0, kt == KF - 1)
                for mt in range(MTL):
                    for nt in range(2):
                        t1 = otmp.tile([P, 512], F32, tag="otmp", name="otmp")
                        nc.vector.tensor_add(
                            t1[:], fps[mt][nt][:],
                            y_sb[:, mt, nt * 512:(nt + 1) * 512],
                        )
                        nc.vector.tensor_add(
                            t1[:], t1[:], b2bc[:, nt * 512:(nt + 1) * 512]
                        )
                        nc.sync.dma_start(
                            io["out"][mt * P:(mt + 1) * P, nt * 512:(nt + 1) * 512],
                            t1[:],
                        )


def build_nc():
    nc = bacc.Bacc(None, target_bir_lowering=False, debug=False, num_devices=NCORES)
    io = {}
    io["x_full"] = nc.dram_tensor("x_full", [T, C], F32, kind="ExternalInput").ap()
    io["xpb"] = nc.dram_tensor("xpb", [P, MTL, C], F32, kind="ExternalInput").ap()
    io["wq"] = nc.dram_tensor("wq", [P, KT, 2, P], F32, kind="ExternalInput").ap()
    io["wk"] = nc.dram_tensor("wk", [P, KT, 2, P], F32, kind="ExternalInput").ap()
    io["wv"] = nc.dram_tensor(
        "wv", [P, KT, NHL * HS], F32, kind="ExternalInput"
    ).ap()
    io["bq"] = nc.dram_tensor("bq", [P, 2], F32, kind="ExternalInput").ap()
    io["bk"] = nc.dram_tensor("bk", [P, 2], F32, kind="ExternalInput").ap()
    io["bv"] = nc.dram_tensor("bv", [NHL * HS], F32, kind="ExternalInput").ap()
    io["wo"] = nc.dram_tensor("wo", [P, KT, C], F32, kind="ExternalInput").ap()
    io["w1"] = nc.dram_tensor("w1", [C, FF], F32, kind="ExternalInput").ap()
    io["b1p"] = nc.dram_tensor("b1p", [P, MFF], F32, kind="ExternalInput").ap()
    io["w2"] = nc.dram_tensor("w2", [FF, C], F32, kind="ExternalInput").ap()
    io["b2"] = nc.dram_tensor("b2", [C], F32, kind="ExternalInput").ap()
    io["out"] = nc.dram_tensor("out", [TLOC, C], F32, kind="ExternalOutput").ap()
    with tile.TileContext(nc) as tc:
        _emit(tc, io)
    nc.compile()
    return nc


def host_prep(inputs):
    """Fold layernorm affines / biases / attention scale into the weights and
    build the 8 per-core input maps."""
    f = np.float32
    x = np.ascontiguousarray(inputs["x"], f)
    Wq, Wk, Wv = (np.asarray(inputs[k], f) for k in ("Wq", "Wk", "Wv"))
    Wo, bo = np.asarray(inputs["Wo"], f), np.asarray(inputs["bo"], f)
    W1, b1 = np.asarray(inputs["W1"], f), np.asarray(inputs["b1"], f)
    W2, b2 = np.asarray(inputs["W2"], f), np.asarray(inputs["b2"], f)
    g1, be1 = np.asarray(inputs["g1"], f), np.asarray(inputs["be1"], f)
    g2, be2 = np.asarray(inputs["g2"], f), np.asarray(inputs["be2"], f)

    scale = f(C) ** f(-0.5)
    Wq_f = (g1[None, :, None] * Wq) * scale
    Wk_f = g1[None, :, None] * Wk
    Wv_f = g1[None, :, None] * Wv
    bq = np.einsum("c,hcd->hd", be1, Wq).astype(f) * scale
    bk = np.einsum("c,hcd->hd", be1, Wk).astype(f)
    bv = np.einsum("c,hcd->hd", be1, Wv).astype(f)
    W1_f = np.ascontiguousarray(g2[:, None] * W1, f)
    b1p = (b1 + be2 @ W1).astype(f)
    Wo_c = np.ascontiguousarray(Wo.reshape(KT, P, C).transpose(1, 0, 2), f)
    W2_c = np.ascontiguousarray(W2, f)
    b1p_dev = np.ascontiguousarray(b1p.reshape(MFF, P).T)

    in_maps = []
    for c in range(NCORES):
        g, r = divmod(c, GSZ)
        hs = [GSZ * r + j for j in range(NHL)]
        wq_pairs = np.stack(
            [np.concatenate([Wq_f[hs[2 * p]], Wq_f[hs[2 * p + 1]]], axis=1)
             for p in range(2)]
        )
        wk_pairs = np.stack(
            [np.concatenate([Wk_f[hs[2 * p]], Wk_f[hs[2 * p + 1]]], axis=1)
             for p in range(2)]
        )
        bq_pairs = np.stack(
            [np.concatenate([bq[hs[2 * p]], bq[hs[2 * p + 1]]]) for p in range(2)]
        )
        bk_pairs = np.stack(
            [np.concatenate([bk[hs[2 * p]], bk[hs[2 * p + 1]]]) for p in range(2)]
        )
        wv_cat = np.concatenate([Wv_f[h] for h in hs], axis=1)
        xpb = x[g, TLOC * r:TLOC * (r + 1)] + bo
        in_maps.append({
            "x_full": np.ascontiguousarray(x[g]),
            "xpb": np.ascontiguousarray(
                xpb.reshape(MTL, P, C).transpose(1, 0, 2)
            ),
            "wq": np.ascontiguousarray(
                wq_pairs.reshape(2, KT, P, P).transpose(2, 1, 0, 3)
            ),
            "wk": np.ascontiguousarray(
                wk_pairs.reshape(2, KT, P, P).transpose(2, 1, 0, 3)
            ),
            "wv": np.ascontiguousarray(
                wv_cat.reshape(KT, P, NHL * HS).transpose(1, 0, 2)
            ),
            "bq": np.ascontiguousarray(bq_pairs.T),
            "bk": np.ascontiguousarray(bk_pairs.T),
            "bv": np.ascontiguousarray(np.concatenate([bv[h] for h in hs])),
            "wo": Wo_c,
            "w1": W1_f,
            "b1p": b1p_dev,
            "w2": W2_c,
            "b2": b2,
        })
    return in_maps


_NC = None


def _get_nc():
    global _NC
    if _NC is None:
        _NC = build_nc()
    return _NC


def kernel(**inputs) -> np.ndarray:
    nc = _get_nc()
    in_maps = host_prep(inputs)
    res = run_bass_kernel_spmd(nc, in_maps, core_ids=list(range(NCORES)))
    out = np.empty((B, T, C), np.float32)
    for c in range(NCORES):
        g, r = divmod(c, GSZ)
        out[g, TLOC * r:TLOC * (r + 1)] = res.results[c]["out"]
    return out
